# revision 8
# baseline (speedup 1.0000x reference)
"""Mixture-of-Experts kernel for Trainium2 (8 NeuronCores).

Strategy (expert-parallel, sparse dispatch — per sharding hint):
  - Host computes the tiny gate (x @ Wg + bg, [16384, 8]), takes top-2,
    softmaxes the two logits, and dispatches tokens by expert id
    (the "all-to-all dispatch tokens by top-k expert id" sharding).
  - Core e receives: its expert's W1/W2/b1 (bf16/f32) and the tokens
    routed to it (transposed, bf16, padded to capacity C). It computes
    gelu(x @ W1 + b1) @ W2 on device (output transposed, ungated).
  - Host applies the per-token gate weights, scatter-adds the
    per-expert outputs into token rows, and adds the gate-weighted b2
    term exactly: out += G @ b2.

Device kernel (per core), all matmuls bf16 with fp32 PSUM accumulation,
two phases with h spilled to a DRAM scratch (h for all C tokens does
not fit in SBUF alongside the weights):

Phase 1 (mm1+gelu), k-outer for LDWEIGHTS amortization:
  for ht: for half: for k: LDW(w1[k,ht]); 4x MM into psum[chunk]
  -> one LDWEIGHTS per 4-5 N=512 matmuls (vs 1:1 column-wise).
  gelu(+b1) drains each bank into h[ht] [128, C] which DMAs to DRAM.
  The 128-token tail rides half B's k-loop on a 9th psum rotation.

Phase 2 (mm2), transposed output, also LDWEIGHTS-amortized:
  stationary = W2 tile [128 h, 128 d], moving = h[ht] 512-token slices
  streamed back from DRAM, output yT[dslice] = [128 d, tok] in PSUM.
  PSUM holds 2 dslices x 4 chunks (2048 tokens) -> one LDWEIGHTS per
  4 N=512 matmuls. h streams from DRAM once per dslice-group (4x
  total; DMA engines have the headroom) on two alternating queues.
  The 128-token tail keeps an h-stationary pass (dt-paired LDWEIGHTS)
  and writes ytail in [tok, D] layout.

W2 is not separately resident: its [128,1024] column slabs DMA into the
same SBUF tiles that held W1 slabs, which die ht-group by ht-group
during phase 1 (tile tag rotation inserts the WAR waits). The prologue
round-robins the PE-critical (w1 slab, x half) pairs over all three
DMA-capable queues (ACT/SP/gpsimd); it is DMA-bandwidth-floor bound.
Duplicate LDWEIGHTS are stripped post-build by _dedup_ldweights (PE
weight state is sticky), leaving ~1.3k loads for ~4.4k matmuls.
"""

import numpy as np
import ml_dtypes

B, M, D, E, TOPK = 4096, 4, 1024, 8, 2
H = 4 * D
N = B * M
P = 128
CT = 512              # tokens per chunk (= one fp32 PSUM bank)
KD = D // P           # 8 k-tiles over D
HT = H // P           # 32 h-tiles over H
SB = 2048             # phase-2 superblock (4 chunks in PSUM)

_BUILD_CACHE = {}


def _build(C, repeat=1):
    """Build + compile the per-core bass program for token capacity C.

    repeat>1 python-unrolls the whole program body `repeat` times (used
    only by timing harnesses to measure steady-state per-iteration cost).
    """
    if (C, repeat) in _BUILD_CACHE:
        return _BUILD_CACHE[(C, repeat)]

    import concourse.mybir as mybir
    import concourse.tile as tile
    from concourse import bacc

    BF = mybir.dt.bfloat16
    F32 = mybir.dt.float32
    GELU = mybir.ActivationFunctionType.Gelu

    NCH = C // CT        # full 512-token chunks (8 for C=4224)
    TAIL = C - NCH * CT  # 128 for C=4224
    NSB = NCH * CT // SB  # 2
    # phase 1 runs two 4-chunk halves over the 8 psum banks; the tail
    # (up to 384 tokens, < 1 bank of fp32) rides a 9th psum rotation
    assert NCH == 8 and TAIL % P == 0 and TAIL <= 384

    nc = bacc.Bacc(trn_type="TRN2", target_bir_lowering=False, debug=False)

    xT = nc.dram_tensor("xT", [KD, P, C], BF, kind="ExternalInput")
    w1 = nc.dram_tensor("w1", [KD, P, H], BF, kind="ExternalInput")
    w2 = nc.dram_tensor("w2", [HT, P, D], BF, kind="ExternalInput")
    b1t = nc.dram_tensor("b1t", [P, HT], F32, kind="ExternalInput")
    yT = nc.dram_tensor("yT", [KD, P, NCH * CT], F32, kind="ExternalOutput")
    ytail = nc.dram_tensor("ytail", [max(TAIL, P), D], F32,
                           kind="ExternalOutput")
    yt_r = ytail.rearrange("(ncs p) d -> ncs p d", p=P)

    with tile.TileContext(nc) as tc:
        with (
            tc.tile_pool(name="wslab", bufs=1) as wp,
            tc.tile_pool(name="xin", bufs=1) as xp,
            tc.tile_pool(name="hout", bufs=2) as hop,
            tc.tile_pool(name="hin", bufs=1) as hip,
            tc.tile_pool(name="yout", bufs=1) as yp,
            tc.tile_pool(name="small", bufs=1) as sp,
            tc.tile_pool(name="hdram", bufs=1, space="DRAM") as dp,
            tc.tile_pool(name="ps", bufs=1, space="PSUM") as pp,
        ):
            for _rep in range(repeat):
                # ---- input loads ---------------------------------------
                # ht=0's k-step needs slab(k, g=0) AND x[k] halfA in
                # lockstep (~0.85us/k of PE work). Round-robin the critical
                # (slab, xA) pairs over all three DMA-capable queues.
                HA = 4 * CT
                queues = [nc.scalar, nc.sync, nc.gpsimd]
                b1sb = sp.tile([P, HT], F32, tag="b1t", name="b1sb")
                xk = [None] * KD
                slab = [[None] * (HT // 8) for _ in range(KD)]
                for k in range(KD):
                    q = queues[k % 3]
                    t = wp.tile([P, 1024], BF, tag=f"sl{k}_0", name=f"w1s{k}_0")
                    q.dma_start(t, w1[k][:, 0:1024])
                    slab[k][0] = t
                    tx = xp.tile([P, C], BF, tag=f"x{k}", name=f"x{k}")
                    q.dma_start(tx[:, 0:HA], xT[k][:, 0:HA])
                    xk[k] = tx
                    if k == 1:
                        nc.sync.dma_start(b1sb, b1t.ap())
                for k in range(KD):
                    queues[k % 3].dma_start(xk[k][:, HA:C], xT[k][:, HA:C])
                if _rep == 0:
                    # Preload the Gelu ACT table while the prologue DMAs run.
                    warm = sp.tile([P, 1], F32, tag="warm", name="warm")
                    nc.scalar.activation(warm, b1sb[:, 0:1], GELU)
                for g in range(1, HT // 8):
                    for k in range(KD):
                        t = wp.tile([P, 1024], BF, tag=f"sl{k}_{g}",
                                    name=f"w1s{k}_{g}")
                        nc.sync.dma_start(t, w1[k][:, 1024 * g:1024 * (g + 1)])
                        slab[k][g] = t

                # ---- phase 1: h[ht] = gelu(W1[:,ht].T @ x + b1), spill --
                # DRAM h scratch: 8 tiles of 4 ht-rows so phase-2 reads only
                # wait on the 4 writes they cover (first reads overlap the
                # phase-1 tail).
                hd = [
                    dp.tile([P, 4, C], BF, tag=f"hd{g2}", name=f"hd{g2}")
                    for g2 in range(HT // 4)
                ]
                # w2 slabs land in dead w1 slab tiles: w2[ht] -> slab tag
                # (k=ht%8, g=ht//8), free after phase-1 ht = 8*(ht//8)+7.
                w2sb = [None] * HT

                for ht in range(HT):
                    g, col = ht // 8, (ht % 8) * P
                    hout = hop.tile([P, C], BF, tag="hout", name="hout")
                    for half in range(2):
                        chunks = range(4 * half, 4 * half + 4)
                        pst = [
                            pp.tile([P, CT], F32, tag=f"ps{c % 4 + 4 * half}",
                                    name=f"ps{c}")
                            for c in chunks
                        ]
                        tail_ps = None
                        if half == 1 and TAIL:
                            tail_ps = pp.tile([P, TAIL], F32, tag="ps0",
                                              name="pstail")
                        for k in range(KD):
                            stat = slab[k][g][:, col:col + P]
                            for i, c in enumerate(chunks):
                                nc.tensor.matmul(
                                    pst[i], stat,
                                    xk[k][:, CT * c:CT * (c + 1)],
                                    start=(k == 0), stop=(k == KD - 1),
                                )
                            if tail_ps is not None:
                                nc.tensor.matmul(
                                    tail_ps, stat,
                                    xk[k][:, NCH * CT:C],
                                    start=(k == 0), stop=(k == KD - 1),
                                )
                        for i, c in enumerate(chunks):
                            nc.scalar.activation(
                                hout[:, CT * c:CT * (c + 1)], pst[i], GELU,
                                bias=b1sb[:, ht:ht + 1],
                            )
                        if tail_ps is not None:
                            nc.scalar.activation(
                                hout[:, NCH * CT:C], tail_ps, GELU,
                                bias=b1sb[:, ht:ht + 1],
                            )
                    nc.gpsimd.dma_start(hd[ht // 4][:, ht % 4], hout)
                    if ht % 8 == 7:
                        for htp in range(8 * (ht // 8), 8 * (ht // 8) + 8):
                            t = wp.tile([P, 1024], BF,
                                        tag=f"sl{htp % 8}_{htp // 8}",
                                        name=f"w2s{htp}")
                            nc.sync.dma_start(t, w2[htp])
                            w2sb[htp] = t

                # ---- phase 2 main: yT[ds] = (h @ W2).T, ratio-4 LDW ----
                # po[j][c] = [128 d, 512 tok], j in {0,1} dslices of the
                # current dgroup, c in 4 chunks of the 2048-token superblock.
                # pass A: D[0:512] via ratio-4 dg-pairs (ds 0-3)
                for sb in range(NSB):
                    for dg in range(2):
                        hp_all = []
                        for ht in range(HT):
                            t = hip.tile(
                                [P, SB], BF, tag=f"hp{ht % 8}",
                                name=f"hp{ht % 8}"
                            )
                            (nc.gpsimd if ht % 2 else nc.sync).dma_start(
                                t,
                                hd[ht // 4][:, ht % 4, sb * SB:(sb + 1) * SB],
                            )
                            hp_all.append(t)
                        po = [
                            [
                                pp.tile([P, CT], F32, tag=f"ps{j * 4 + c}",
                                        name=f"po{j}_{c}")
                                for c in range(4)
                            ]
                            for j in range(2)
                        ]
                        for ht in range(HT):
                            hpt = hp_all[ht]
                            for j in range(2):
                                ds = dg * 2 + j
                                stat = w2sb[ht][:, ds * P:(ds + 1) * P]
                                for c in range(4):
                                    nc.tensor.matmul(
                                        po[j][c], stat,
                                        hpt[:, CT * c:CT * (c + 1)],
                                        start=(ht == 0), stop=(ht == HT - 1),
                                    )
                        for j in range(2):
                            ds = dg * 2 + j
                            for c in range(4):
                                ysb = yp.tile([P, CT], F32,
                                              tag=f"y{j * 4 + c}",
                                              name=f"ysb{j}_{c}")
                                nc.vector.tensor_copy(ysb, po[j][c])
                                nc.scalar.dma_start(
                                    yT[ds][:, sb * SB + CT * c:
                                           sb * SB + CT * (c + 1)],
                                    ysb,
                                )

                # pass B: D[512:1024] via ratio-2 4ds x 2ch blocks of
                # 1024 tokens (halves h restream vs running dg 2-3)
                for blk in range(NCH * CT // 1024):
                    hb_all = []
                    for ht in range(HT):
                        t = hip.tile(
                            [P, 1024], BF, tag=f"hp{ht % 8}",
                            name=f"hpb{ht % 8}"
                        )
                        (nc.gpsimd if ht % 2 else nc.sync).dma_start(
                            t,
                            hd[ht // 4][:, ht % 4,
                                        1024 * blk:1024 * (blk + 1)],
                        )
                        hb_all.append(t)
                    pob = [
                        [
                            pp.tile([P, CT], F32, tag=f"ps{j * 2 + c}",
                                    name=f"pob{j}_{c}")
                            for c in range(2)
                        ]
                        for j in range(4)
                    ]
                    for ht in range(HT):
                        hpt = hb_all[ht]
                        for j in range(4):
                            ds = 4 + j
                            stat = w2sb[ht][:, ds * P:(ds + 1) * P]
                            for c in range(2):
                                nc.tensor.matmul(
                                    pob[j][c], stat,
                                    hpt[:, CT * c:CT * (c + 1)],
                                    start=(ht == 0), stop=(ht == HT - 1),
                                )
                    for j in range(4):
                        ds = 4 + j
                        for c in range(2):
                            ysb = yp.tile([P, CT], F32,
                                          tag=f"y{j * 2 + c}",
                                          name=f"ysbb{j}_{c}")
                            nc.vector.tensor_copy(ysb, pob[j][c])
                            nc.scalar.dma_start(
                                yT[ds][:, 1024 * blk + CT * c:
                                       1024 * blk + CT * (c + 1)],
                                ysb,
                            )

                # ---- phase 2 tail: h-stationary, dt-paired LDW ---------
                if TAIL:
                    ncs = TAIL // P
                    hq = []
                    for g2 in range(HT // 4):
                        t = hip.tile([P, 4, TAIL], BF, tag=f"hp{g2}",
                                     name=f"hq{g2}")
                        nc.gpsimd.dma_start(t, hd[g2][:, :, NCH * CT:C])
                        hq.append(t)
                    pot = [
                        [
                            pp.tile([P, CT], F32, tag=f"ps{cs * 2 + dt}",
                                    name=f"pot{cs}_{dt}")
                            for dt in range(2)
                        ]
                        for cs in range(ncs)
                    ]
                    for ht in range(HT):
                        hpt = hq[ht // 4][:, ht % 4]
                        for cs in range(ncs):
                            stat = hpt[:, cs * P:(cs + 1) * P]
                            for dt in range(2):
                                nc.tensor.matmul(
                                    pot[cs][dt], stat,
                                    w2sb[ht][:, 512 * dt:512 * (dt + 1)],
                                    start=(ht == 0), stop=(ht == HT - 1),
                                )
                    for cs in range(ncs):
                        for dt in range(2):
                            ysb = yp.tile([P, CT], F32, tag=f"y{cs * 2 + dt}",
                                          name=f"ytl{cs}_{dt}")
                            nc.vector.tensor_copy(ysb, pot[cs][dt])
                            nc.scalar.dma_start(
                                yt_r[cs][:, 512 * dt:512 * (dt + 1)], ysb,
                            )
    _dedup_ldweights(nc)
    nc.compile()
    _BUILD_CACHE[(C, repeat)] = nc
    return nc


def _ap_key(arg):
    """Stable identity key for an instruction AP argument, or None."""
    try:
        ap = arg.bass_ap if hasattr(arg, "bass_ap") else arg
        t = ap.tensor
        return (t.name, ap.offset, tuple(map(tuple, ap.ap)))
    except Exception:
        return None


def _dedup_ldweights(nc):
    """Drop an InstLdweights when the immediately-preceding PE instruction
    sequence already loaded the identical weights AP (PE weight state is
    sticky until the next LDWEIGHTS). Only sync-free duplicates are dropped.
    """
    import concourse.mybir as mybir

    n_del = 0
    for blk in nc.m.functions[0].blocks:
        insts = list(blk.instructions)
        keep = []
        last_key = None
        for inst in insts:
            tn = type(inst).__name__
            if tn == "InstLdweights":
                key = _ap_key(inst.ins[0])
                si = inst.sync_info
                clean = not (si and (si.on_wait or si.on_update))
                if key is not None and key == last_key and clean:
                    n_del += 1
                    continue
                last_key = key
            elif tn != "InstMatmult" and getattr(inst, "engine", None) == mybir.EngineType.PE:
                last_key = None
            keep.append(inst)
        if len(keep) != len(insts):
            while len(blk.instructions):
                blk.instructions.pop()
            for inst in keep:
                blk.instructions.append(inst)
    return n_del


def _route(xf, Wg, bg):
    """Top-2 gating on host. Returns (idx, gate) per expert and dense G."""
    logits = xf @ Wg + bg                      # [N, E] f32
    n = logits.shape[0]
    ar = np.arange(n)
    i1 = np.argmax(logits, axis=1)
    v1 = logits[ar, i1]
    masked = logits.copy()
    masked[ar, i1] = -np.inf
    i2 = np.argmax(masked, axis=1)
    v2 = masked[ar, i2]
    e2 = np.exp(v2 - v1)
    wt1 = 1.0 / (1.0 + e2)
    wt2 = e2 / (1.0 + e2)
    G = np.zeros_like(logits)
    G[ar, i1] = wt1
    G[ar, i2] = wt2
    idxs, gates = [], []
    for e in range(E):
        idx = np.nonzero((i1 == e) | (i2 == e))[0]
        idxs.append(idx)
        gates.append(G[idx, e].astype(np.float32))
    return idxs, gates, G.astype(np.float32)


def kernel(_trace=False, **inputs):
    x = np.asarray(inputs["x"], dtype=np.float32)
    Wg = np.asarray(inputs["Wg"], dtype=np.float32)
    bg = np.asarray(inputs["bg"], dtype=np.float32)
    W1 = np.asarray(inputs["W1"], dtype=np.float32)
    b1 = np.asarray(inputs["b1"], dtype=np.float32)
    W2 = np.asarray(inputs["W2"], dtype=np.float32)
    b2 = np.asarray(inputs["b2"], dtype=np.float32)

    Bn, Mn, Dn = x.shape
    n = Bn * Mn
    xf = x.reshape(n, Dn)

    idxs, gates, G = _route(xf, Wg, bg)

    C = max(len(i) for i in idxs)
    C = ((C + P - 1) // P) * P
    # the device program is specialized for 8 full chunks + 128 tail
    C = max(C, 8 * CT + P)

    bf16 = ml_dtypes.bfloat16
    xf_bf = xf.astype(bf16)

    in_maps = []
    for e in range(E):
        ne = len(idxs[e])
        xTe = np.zeros((Dn, C), dtype=bf16)
        xTe[:, :ne] = xf_bf[idxs[e]].T
        in_maps.append({
            "xT": np.ascontiguousarray(xTe.reshape(KD, P, C)),
            "w1": np.ascontiguousarray(W1[e].astype(bf16).reshape(KD, P, H)),
            "w2": np.ascontiguousarray(W2[e].astype(bf16).reshape(HT, P, D)),
            "b1t": np.ascontiguousarray(b1[e].reshape(HT, P).T),
        })

    nc = _build(C)

    from concourse.bass_utils import run_bass_kernel_spmd
    res = run_bass_kernel_spmd(
        nc, in_maps, core_ids=list(range(E)), trace=_trace
    )

    out = G @ b2                               # gate-weighted b2, exact
    NM = 8 * CT                                # main tokens (yT layout)
    for e in range(E):
        ne = len(idxs[e])
        ye = np.empty((ne, Dn), dtype=np.float32)
        nm = min(ne, NM)
        # yT is [dslice, d, tok] with D-index = dslice*128 + d
        ye[:nm] = res.results[e]["yT"].reshape(Dn, NM)[:, :nm].T
        if ne > NM:
            ye[NM:] = res.results[e]["ytail"][:ne - NM]
        out[idxs[e]] += gates[e][:, None] * ye

    if _trace:
        return out.reshape(Bn, Mn, Dn), res
    return out.reshape(Bn, Mn, Dn)
